# revision 17
# baseline (speedup 1.0000x reference)
"""GATv2 layer kernel for Trainium2 (Bass/Tile), 8-core SPMD — v6.

Problem (hardcoded): B=4, N=512, D=128, H=8 heads, F=16 hidden, is_concat.
  g_l = h @ W_l.T ; g_r = h @ W_r.T               [B,N,H,F]
  e[b,i,j,h] = sum_f a_w[f]*lrelu(g_l[b,j,h,f] + g_r[b,i,h,f], 0.2)
  a = softmax_j(e masked by adj)                  [B,i,j,H]
  out[b,i,h,f] = sum_j a[b,i,j,h]*g_r[b,j,h,f]   -> [B,N,H*F]

Sharding: 8 cores = (batch b) x (i-half); 256 target nodes per core with
fully-replicated g_l/g_r.

Per-core math (rows (il*8+h), 16 groups of 16 i's):
  lrelu(x) = 0.8*relu(x) + 0.2*x; per-(i,h) j-constant terms cancel in
  softmax, so
    e[(il,h), j] = 0.8*sum_hf Aaw[hf,h]*relu(g_lT[hf,j] + g_rT[hf,i])
                   + 0.2*alT[h,j] - 44*(1-adj[i,j])    (+ const per row)
  The alT linear term AND the adjacency mask are applied by one K=24
  accumulating matmul (rhs = 16 maskinv rows + 8 alT rows); the -44
  penalty underflows exp() to exact 0 in fp16 (no post-exp mask multiply).

Software-pipelined by pairs: stage p emits exp for pair p-1, the
transpose/copy/agg for pair p-2, the score stage for pair p, then the
norm/out tail for pair p-2 — so no strict-FIFO engine queue waits at its
head on same-stage cross-engine work.

Score stage per group:
  t[hf,j] = relu(g_lT + g_rT[:,i]) - 12 on DVE (dual-op tensor_scalar,
            ~262ns) + 4 on ACT (activation relu+bias, ~620ns)
  e2      = 16 M=32 matmuls on 4 concurrent PE column strips
            (tile_position=(0,32s), per-strip start) + the K=24 mask/alT
            matmul last (carries the stop flag)
  um      = one exp over the pair's [128,1024] PSUM -> fp16       ACT
Per pair of groups:
  umT     = 8 PE transposes -> one fp16 PSUM bank [128,1024]
  umt_sb  = one ACT copy PSUM->SBUF
  aggT    = umt.T @ g_r_nat (4 K-chunks, ones col -> den)        PE
  agg_sb  = aggT * (1/den) * headmask                            DVE
  out     = R.T @ agg_sb (4 groups per 64-row stripe)            PE
"""

import ml_dtypes
import numpy as np
from contextlib import ExitStack

import concourse.bass as bass
import concourse.bacc as bacc
import concourse.tile as tile
import concourse.mybir as mybir
from concourse.bass_utils import run_bass_kernel_spmd

B, N, D = 4, 512, 128
H, F = 8, 16
NEG_SLOPE = 0.2
NCORES = 8
IHALF = N // 2          # 256 target nodes per core
GSIZE = 16              # i's per group
NGROUPS = IHALF // GSIZE  # 16
MASKPEN = -44.0
f32 = mybir.dt.float32
f16 = mybir.dt.float16

# Engine per relu slot (program order = strip-round-robin il order):
# V = VectorE dual-op tensor_scalar, A = ScalarE activation relu.
ENG_SCHED = "AVVVAVVVAVVVAVVV"


def build_program():
    nc = bacc.Bacc(
        "TRN2", target_bir_lowering=False, debug=False, num_devices=NCORES
    )

    d_hT = nc.dram_tensor("hT", [D, N], f16, kind="ExternalInput").ap()
    d_WlT = nc.dram_tensor("WlT", [D, H * F], f16, kind="ExternalInput").ap()
    d_WrT = nc.dram_tensor("WrT", [D, H * F], f16, kind="ExternalInput").ap()
    # Amask[:, 24:32] = 0.8*Aaw; LDW windows [24-8m, 56-8m) place node
    # (il%4)'s head-block at strip rows 8m..8m+8.
    d_Aaw = nc.dram_tensor("Amask", [H * F, 56], f16, kind="ExternalInput").ap()
    d_Aaw02 = nc.dram_tensor("Aaw02", [H * F, H], f16, kind="ExternalInput").ap()
    d_m24w = nc.dram_tensor("m24w", [24, 128], f16, kind="ExternalInput").ap()
    d_minv = nc.dram_tensor("maskinv", [GSIZE, NGROUPS * N], f16,
                            kind="ExternalInput").ap()
    d_R = nc.dram_tensor("Rmask", [128, 4 * 64], f16, kind="ExternalInput").ap()
    d_hm = nc.dram_tensor("headmask", [128, H * F], f32, kind="ExternalInput").ap()
    d_id = nc.dram_tensor("ident", [128, 128], f16, kind="ExternalInput").ap()
    d_out = nc.dram_tensor("out", [IHALF, D], f32, kind="ExternalOutput").ap()

    with tile.TileContext(nc) as tc:
        with ExitStack() as ctx:
            _gat_body(ctx, tc, d_out, d_hT, d_WlT, d_WrT, d_Aaw, d_Aaw02,
                      d_m24w, d_minv, d_R, d_hm, d_id)
    nc.compile()
    return nc


def _gat_body(ctx, tc, d_out, d_hT, d_WlT, d_WrT, d_Aaw, d_Aaw02, d_m24w,
              d_minv, d_R, d_hm, d_id):
    nc = tc.nc
    add = mybir.AluOpType.add
    amax = mybir.AluOpType.max
    mult = mybir.AluOpType.mult
    Relu = mybir.ActivationFunctionType.Relu
    Exp = mybir.ActivationFunctionType.Exp

    consts = ctx.enter_context(tc.tile_pool(name="consts", bufs=1))
    tpool = ctx.enter_context(tc.tile_pool(name="tpool", bufs=20))
    umpool = ctx.enter_context(tc.tile_pool(name="umpool", bufs=3))
    umtp = ctx.enter_context(tc.tile_pool(name="umtp", bufs=3))
    aggp = ctx.enter_context(tc.tile_pool(name="aggp", bufs=3))
    denp = ctx.enter_context(tc.tile_pool(name="denp", bufs=3))
    outp = ctx.enter_context(tc.tile_pool(name="outp", bufs=2))

    # PSUM budget (8 banks): ppe 2x[128,1024]f32 (4) + pumt 1x[128,1024]f16
    # (1) + pagg 2x[128,258]f32 (2) + pout 1x[128,128]f32 (1)
    ppe = ctx.enter_context(tc.tile_pool(name="ppe", bufs=2, space="PSUM"))
    pumt = ctx.enter_context(tc.tile_pool(name="pumt", bufs=1, space="PSUM"))
    pagg = ctx.enter_context(tc.tile_pool(name="pagg", bufs=1, space="PSUM"))
    pout = ctx.enter_context(tc.tile_pool(name="pout", bufs=1, space="PSUM"))

    # ---- ACT table warm-up (overlaps the input DMAs) ----
    warm = consts.tile([128, 8], f32, tag="warm")
    nc.vector.memset(warm[:], 0.0)
    nc.scalar.activation(warm[:], warm[:], Exp)

    # ---- load constants ----
    s_hT = consts.tile([D, N], f16, tag="ht")
    nc.sync.dma_start(out=s_hT[:], in_=d_hT)
    s_WlT = consts.tile([D, H * F], f16, tag="wlt")
    nc.sync.dma_start(out=s_WlT[:], in_=d_WlT)
    s_WrT = consts.tile([D, H * F], f16, tag="wrt")
    nc.sync.dma_start(out=s_WrT[:], in_=d_WrT)
    s_Aaw = consts.tile([H * F, 56], f16, tag="aaw")
    nc.scalar.dma_start(out=s_Aaw[:], in_=d_Aaw)
    s_Aaw02 = consts.tile([H * F, H], f16, tag="aaw02")
    nc.scalar.dma_start(out=s_Aaw02[:], in_=d_Aaw02)
    s_m24w = consts.tile([24, 128], f16, tag="m24w")
    nc.scalar.dma_start(out=s_m24w[:], in_=d_m24w)
    s_R = consts.tile([128, 4 * 64], f16, tag="rmat")
    nc.sync.dma_start(out=s_R[:], in_=d_R)
    s_hm = consts.tile([128, H * F], f32, tag="hm")
    nc.sync.dma_start(out=s_hm[:], in_=d_hm)
    s_id = consts.tile([128, 128], f16, tag="ident")
    nc.sync.dma_start(out=s_id[:], in_=d_id)
    # K=24 rhs for all groups: rows 0:16 maskinv, rows 16:24 alT02 replicated
    m24 = consts.tile([24, NGROUPS * N], f16, tag="m24")
    nc.sync.dma_start(out=m24[0:GSIZE, :], in_=d_minv)

    # ---- setup: projections ----
    # g_lT kept twice: f16 SBUF (DVE relu src, alT02 rhs) and fp32 PSUM
    # (ACT relu src: ScalarE's PSUM port is faster than its SBUF port)
    g_lT = consts.tile([H * F, N], f16, tag="glt")
    ps = ppe.tile([128, 2 * N], f32, tag="pe2")
    nc.tensor.matmul(ps[:, 0:N], s_WlT[:], s_hT[:], start=True, stop=True)
    nc.scalar.copy(g_lT[:], ps[:, 0:N])

    g_rT32 = consts.tile([H * F, N], f32, tag="grt32")
    ps = ppe.tile([128, 2 * N], f32, tag="pe2")
    nc.tensor.matmul(ps[:, 0:N], s_WrT[:], s_hT[:], start=True, stop=True)
    nc.vector.tensor_copy(g_rT32[:], ps[:, 0:N])

    # g_r natural layout + ones column per 128-row chunk (den for free)
    g_r_nat = consts.tile([128, 4 * 129], f16, tag="grnat")
    for c in range(4):
        cs = slice(128 * c, 128 * (c + 1))
        pq = pagg.tile([128, 258], f32, tag="agg")
        nc.tensor.matmul(pq[:, 0:128], s_hT[:, cs], s_WrT[:],
                         start=True, stop=True)
        nc.vector.tensor_copy(g_r_nat[:, 129 * c:129 * c + 128], pq[:, 0:128])
        nc.vector.memset(g_r_nat[:, 129 * c + 128:129 * c + 129], 1.0)

    # alT02[h,j] = 0.2*sum_f a_w[f]*g_l[j,(h,f)]; replicate x16 into m24 rows
    alT02 = consts.tile([H, N], f16, tag="alt02")
    ps = ppe.tile([128, 2 * N], f32, tag="pe2")
    nc.tensor.matmul(ps[0:H, 0:N], s_Aaw02[:], g_lT[:], start=True, stop=True)
    nc.scalar.copy(alT02[:], ps[0:H, 0:N])
    rep_ap = bass.AP(alT02.tensor, alT02[:].offset,
                     [alT02[:].ap[0], [0, NGROUPS], [1, N]])
    nc.sync.dma_start(out=m24[GSIZE:24, :], in_=rep_ap)

    # ---- main loop: software-pipelined pairs of 16-node groups ----
    # Stage p emits: exp for pair p-1 (ACT), transpose/copy/agg for pair
    # p-2 (PE+ACT), the score stage for pair p (DVE/ACT relus + PE MMs),
    # then recip/stt/R/out for pair p-2 (DVE+PE tails). Each engine's
    # strict-FIFO queue therefore never waits at its head on work another
    # engine issued in the same stage.
    order = [4 * s + r for r in range(4) for s in range(4)]
    NPAIRS = NGROUPS // 2
    e2s = {}
    um2s = {}
    umts = {}
    aggs = {}
    state = {"out_ps": None}

    def stage_exp(p):
        um2 = umpool.tile([128, 2 * N], f16, tag="um")
        nc.scalar.activation(um2[:], e2s.pop(p)[:], Exp)
        um2s[p] = um2

    def stage_transpose(p):
        um2 = um2s.pop(p)
        umt_ps = pumt.tile([128, 2 * N], f16, tag="umt")
        for k in range(8):
            ks = slice(128 * k, 128 * (k + 1))
            nc.tensor.transpose(umt_ps[:, ks], um2[:, ks], s_id[:])
        umt = umtp.tile([128, 2 * N], f16, tag="umtsb")
        nc.scalar.copy(umt[:], umt_ps[:])
        umts[p] = umt

    def stage_agg(p):
        umt = umts.pop(p)
        agg_ps = pagg.tile([128, 258], f32, tag="agg")
        for gg in range(2):
            for c in range(4):
                nc.tensor.matmul(agg_ps[:, 129 * gg:129 * gg + 129],
                                 umt[:, 512 * gg + 128 * c:
                                        512 * gg + 128 * (c + 1)],
                                 g_r_nat[:, 129 * c:129 * c + 129],
                                 start=(c == 0), stop=(c == 3))
        aggs[p] = agg_ps

    def stage_scores(p):
        e2 = ppe.tile([128, 2 * N], f32, tag="pe2")
        for gg in range(2):
            g = 2 * p + gg
            off = N * gg
            for idx, il in enumerate(order):
                i = GSIZE * g + il
                t_t = tpool.tile([H * F, N], f16, tag="t")
                if ENG_SCHED[idx] == "V":
                    nc.vector.tensor_scalar(t_t[:], g_lT[:],
                                            g_rT32[:, i:i + 1], 0.0, add, amax)
                else:
                    nc.scalar.activation(t_t[:], g_lT[:], Relu,
                                         bias=g_rT32[:, i:i + 1], scale=1.0)
                s = il // 4
                w0 = 24 - 8 * (il % 4)
                nc.tensor.matmul(e2[32 * s:32 * s + 32, off:off + N],
                                 s_Aaw[:, w0:w0 + 32], t_t[:],
                                 start=(idx < 4), stop=False,
                                 tile_position=(0, 32 * s),
                                 skip_group_check=True)
            # mask + alT linear term last: one K=24 matmul over all rows;
            # carries the stop flag so exp depends only on it
            nc.tensor.matmul(e2[:, off:off + N], s_m24w[:],
                             m24[:, N * g:N * (g + 1)],
                             start=False, stop=True, skip_group_check=True)
        e2s[p] = e2

    def stage_norm_out(p):
        agg_ps = aggs.pop(p)
        rd2 = denp.tile([128, 2], f32, tag="rden")
        den_ap = bass.AP(agg_ps.tensor, agg_ps[:, 128:129].offset,
                         [agg_ps[:, 128:129].ap[0], [129, 2], [1, 1]])
        nc.vector.reciprocal(rd2[:], den_ap)
        for gg in range(2):
            g = 2 * p + gg
            agg_sb = aggp.tile([128, D], f16, tag="aggsb")
            nc.vector.scalar_tensor_tensor(
                agg_sb[:], agg_ps[:, 129 * gg:129 * gg + 128],
                rd2[:, gg:gg + 1], s_hm[:], mult, mult)
            if g % 8 == 0:
                out_ps_new = pout.tile([128, D], f32, tag="out")
                state["out_ps"] = out_ps_new
            q = g % 4
            stripe = 64 * ((g % 8) // 4)
            out_ps = state["out_ps"]
            nc.tensor.matmul(out_ps[stripe:stripe + 64, :],
                             s_R[:, 64 * q:64 * q + 64], agg_sb[:],
                             start=(q == 0), stop=(q == 3))
            if q == 3:
                outb = outp.tile([64, D], f32, tag="outb")
                nc.scalar.copy(outb[:], out_ps[stripe:stripe + 64, :])
                r0 = 64 * (g // 4)
                nc.sync.dma_start(out=d_out[r0:r0 + 64, :], in_=outb[:])

    for p in range(NPAIRS + 3):
        if p >= 1 and p - 1 < NPAIRS:
            stage_exp(p - 1)
        if p >= 2 and p - 2 < NPAIRS:
            stage_transpose(p - 2)
        if p >= 3:
            stage_agg(p - 3)
        if p < NPAIRS:
            stage_scores(p)
        if p >= 3:
            stage_norm_out(p - 3)


def _host_inputs(h, adj, W_l, W_r, a_w):
    """Build the per-core input maps (pure layout/constant prep)."""
    HF = H * F
    Aaw = np.zeros((HF, H), dtype=np.float32)
    for hh in range(H):
        Aaw[hh * F:(hh + 1) * F, hh] = a_w
    Amask = np.zeros((HF, 56), dtype=np.float32)
    Amask[:, 24:32] = (1.0 - NEG_SLOPE) * Aaw
    Amask = Amask.astype(np.float16)
    Aaw02 = (NEG_SLOPE * Aaw).astype(np.float16)
    m24w = np.zeros((24, 128), dtype=np.float32)
    for il in range(GSIZE):
        for hh in range(H):
            m24w[il, il * H + hh] = MASKPEN
            m24w[16 + hh, il * H + hh] = 1.0
    m24w = m24w.astype(np.float16)
    Rmask = np.zeros((128, 4 * 64), dtype=np.float16)
    for q in range(4):
        for il in range(GSIZE):
            Rmask[il * H:(il + 1) * H, 64 * q + 16 * q + il] = 1.0
    headmask = np.zeros((128, HF), dtype=np.float32)
    for il in range(GSIZE):
        for hh in range(H):
            headmask[il * H + hh, hh * F:(hh + 1) * F] = 1.0
    ident = np.eye(128, dtype=np.float16)
    WlT = np.ascontiguousarray(W_l.T).astype(np.float16)
    WrT = np.ascontiguousarray(W_r.T).astype(np.float16)

    in_maps = []
    for c in range(NCORES):
        b = c // 2
        i0 = IHALF * (c % 2)
        # Roll the node axis so this core's targets sit at 0..IHALF-1;
        # source order permuted consistently (softmax/agg j-invariant).
        maskseg = np.roll(adj[b, i0:i0 + IHALF, :, 0], -i0, axis=1)
        maskinv = np.ascontiguousarray(
            (1.0 - maskseg.reshape(NGROUPS, GSIZE, N).transpose(1, 0, 2)
             ).reshape(GSIZE, NGROUPS * N)).astype(np.float16)
        in_maps.append({
            "hT": np.ascontiguousarray(np.roll(h[b], -i0, axis=0).T).astype(
                np.float16),
            "WlT": WlT,
            "WrT": WrT,
            "Amask": Amask,
            "Aaw02": Aaw02,
            "m24w": m24w,
            "maskinv": maskinv,
            "Rmask": Rmask,
            "headmask": headmask,
            "ident": ident,
        })
    return in_maps


_NC_CACHE = {}
LAST_RESULT = None  # BassKernelResults of the most recent kernel() call


def _get_program():
    if "nc" not in _NC_CACHE:
        _NC_CACHE["nc"] = build_program()
    return _NC_CACHE["nc"]


def kernel(h, adj, W_l, W_r, a_w):
    h = np.asarray(h)
    adj = np.asarray(adj)
    W_l = np.asarray(W_l)
    W_r = np.asarray(W_r)
    a_w = np.asarray(a_w)

    nc = _get_program()
    in_maps = _host_inputs(h, adj, W_l, W_r, a_w)
    res = None
    for attempt in range(3):
        try:
            res = run_bass_kernel_spmd(nc, in_maps, list(range(NCORES)))
            break
        except Exception:
            # the axon-proxied device occasionally reports a transient
            # "unrecoverable" state at process start; it self-heals
            if attempt == 2:
                raise
            import time
            time.sleep(20)
    global LAST_RESULT
    LAST_RESULT = res

    out = np.zeros((B, N, D), dtype=np.float32)
    for c in range(NCORES):
        b = c // 2
        i0 = IHALF * (c % 2)
        out[b, i0:i0 + IHALF, :] = res.results[c]["out"]
    return out


# revision 19
# speedup vs baseline: 1.0196x; 1.0196x over previous
"""GATv2 layer kernel for Trainium2 (Bass/Tile), 8-core SPMD — v6.

Problem (hardcoded): B=4, N=512, D=128, H=8 heads, F=16 hidden, is_concat.
  g_l = h @ W_l.T ; g_r = h @ W_r.T               [B,N,H,F]
  e[b,i,j,h] = sum_f a_w[f]*lrelu(g_l[b,j,h,f] + g_r[b,i,h,f], 0.2)
  a = softmax_j(e masked by adj)                  [B,i,j,H]
  out[b,i,h,f] = sum_j a[b,i,j,h]*g_r[b,j,h,f]   -> [B,N,H*F]

Sharding: 8 cores = (batch b) x (i-half); 256 target nodes per core with
fully-replicated g_l/g_r.

Per-core math (rows (il*8+h), 16 groups of 16 i's):
  lrelu(x) = 0.8*relu(x) + 0.2*x; per-(i,h) j-constant terms cancel in
  softmax, so
    e[(il,h), j] = 0.8*sum_hf Aaw[hf,h]*relu(g_lT[hf,j] + g_rT[hf,i])
                   + 0.2*alT[h,j] - 44*(1-adj[i,j])    (+ const per row)
  The alT linear term AND the adjacency mask are applied by one K=24
  accumulating matmul (rhs = 16 maskinv rows + 8 alT rows); the -44
  penalty underflows exp() to exact 0 in fp16 (no post-exp mask multiply).

Software-pipelined by pairs: stage p emits exp for pair p-1, the
transpose/copy/agg for pair p-2, the score stage for pair p, then the
norm/out tail for pair p-2 — so no strict-FIFO engine queue waits at its
head on same-stage cross-engine work.

Score stage per group:
  t[hf,j] = relu(g_lT + g_rT[:,i]) - 12 on DVE (dual-op tensor_scalar,
            ~262ns) + 4 on ACT (activation relu+bias, ~620ns)
  e2      = 16 M=32 matmuls on 4 concurrent PE column strips
            (tile_position=(0,32s), per-strip start) + the K=24 mask/alT
            matmul last (carries the stop flag)
  um      = one exp over the pair's [128,1024] PSUM -> fp16       ACT
Per pair of groups:
  umT     = 8 PE transposes -> one fp16 PSUM bank [128,1024]
  umt_sb  = one ACT copy PSUM->SBUF
  aggT    = umt.T @ g_r_nat (4 K-chunks, ones col -> den)        PE
  agg_sb  = aggT * (1/den) * headmask                            DVE
  out     = R.T @ agg_sb (4 groups per 64-row stripe)            PE
"""

import ml_dtypes
import numpy as np
from contextlib import ExitStack

import concourse.bass as bass
import concourse.bacc as bacc
import concourse.tile as tile
import concourse.mybir as mybir
from concourse.bass_utils import run_bass_kernel_spmd

B, N, D = 4, 512, 128
H, F = 8, 16
NEG_SLOPE = 0.2
NCORES = 8
IHALF = N // 2          # 256 target nodes per core
GSIZE = 16              # i's per group
NGROUPS = IHALF // GSIZE  # 16
MASKPEN = -44.0
f32 = mybir.dt.float32
f16 = mybir.dt.float16

# Engine per relu slot (program order = strip-round-robin il order):
# V = VectorE dual-op tensor_scalar, A = ScalarE activation relu.
ENG_SCHED = "AVVVAVVVAVVVAVVV"


def build_program():
    nc = bacc.Bacc(
        "TRN2", target_bir_lowering=False, debug=False, num_devices=NCORES
    )

    d_hT = nc.dram_tensor("hT", [D, N], f16, kind="ExternalInput").ap()
    d_WlT = nc.dram_tensor("WlT", [D, H * F], f16, kind="ExternalInput").ap()
    d_WrT = nc.dram_tensor("WrT", [D, H * F], f16, kind="ExternalInput").ap()
    # Amask[:, 24:32] = 0.8*Aaw; LDW windows [24-8m, 56-8m) place node
    # (il%4)'s head-block at strip rows 8m..8m+8.
    d_Aaw = nc.dram_tensor("Amask", [H * F, 56], f16, kind="ExternalInput").ap()
    d_Aaw02 = nc.dram_tensor("Aaw02", [H * F, H], f16, kind="ExternalInput").ap()
    d_m24w = nc.dram_tensor("m24w", [24, 128], f16, kind="ExternalInput").ap()
    d_minv = nc.dram_tensor("maskinv", [GSIZE, NGROUPS * N], f16,
                            kind="ExternalInput").ap()
    d_R = nc.dram_tensor("Rmask", [128, 4 * 64], f16, kind="ExternalInput").ap()
    d_hm = nc.dram_tensor("headmask", [128, H * F], f32, kind="ExternalInput").ap()
    d_id = nc.dram_tensor("ident", [128, 128], f16, kind="ExternalInput").ap()
    d_out = nc.dram_tensor("out", [IHALF, D], f32, kind="ExternalOutput").ap()

    with tile.TileContext(nc) as tc:
        with ExitStack() as ctx:
            _gat_body(ctx, tc, d_out, d_hT, d_WlT, d_WrT, d_Aaw, d_Aaw02,
                      d_m24w, d_minv, d_R, d_hm, d_id)
    nc.compile()
    return nc


def _gat_body(ctx, tc, d_out, d_hT, d_WlT, d_WrT, d_Aaw, d_Aaw02, d_m24w,
              d_minv, d_R, d_hm, d_id):
    nc = tc.nc
    add = mybir.AluOpType.add
    amax = mybir.AluOpType.max
    mult = mybir.AluOpType.mult
    Relu = mybir.ActivationFunctionType.Relu
    Exp = mybir.ActivationFunctionType.Exp

    consts = ctx.enter_context(tc.tile_pool(name="consts", bufs=1))
    tpool = ctx.enter_context(tc.tile_pool(name="tpool", bufs=24))
    umpool = ctx.enter_context(tc.tile_pool(name="umpool", bufs=4))
    umtp = ctx.enter_context(tc.tile_pool(name="umtp", bufs=3))
    aggp = ctx.enter_context(tc.tile_pool(name="aggp", bufs=4))
    denp = ctx.enter_context(tc.tile_pool(name="denp", bufs=4))
    outp = ctx.enter_context(tc.tile_pool(name="outp", bufs=2))

    # PSUM budget (8 banks): ppe 2x[128,1024]f32 (4) + pumt 1x[128,1024]f16
    # (1) + pagg 2x[128,258]f32 (2) + pout 1x[128,128]f32 (1)
    ppe = ctx.enter_context(tc.tile_pool(name="ppe", bufs=2, space="PSUM"))
    pumt = ctx.enter_context(tc.tile_pool(name="pumt", bufs=1, space="PSUM"))
    pagg = ctx.enter_context(tc.tile_pool(name="pagg", bufs=1, space="PSUM"))
    pout = ctx.enter_context(tc.tile_pool(name="pout", bufs=1, space="PSUM"))

    # ---- ACT table warm-up (overlaps the input DMAs) ----
    warm = consts.tile([128, 8], f32, tag="warm")
    nc.vector.memset(warm[:], 0.0)
    nc.scalar.activation(warm[:], warm[:], Exp)

    # ---- load constants ----
    s_hT = consts.tile([D, N], f16, tag="ht")
    nc.sync.dma_start(out=s_hT[:], in_=d_hT)
    s_WlT = consts.tile([D, H * F], f16, tag="wlt")
    nc.sync.dma_start(out=s_WlT[:], in_=d_WlT)
    s_WrT = consts.tile([D, H * F], f16, tag="wrt")
    nc.sync.dma_start(out=s_WrT[:], in_=d_WrT)
    s_Aaw = consts.tile([H * F, 56], f16, tag="aaw")
    nc.scalar.dma_start(out=s_Aaw[:], in_=d_Aaw)
    s_Aaw02 = consts.tile([H * F, H], f16, tag="aaw02")
    nc.scalar.dma_start(out=s_Aaw02[:], in_=d_Aaw02)
    s_m24w = consts.tile([24, 128], f16, tag="m24w")
    nc.scalar.dma_start(out=s_m24w[:], in_=d_m24w)
    s_R = consts.tile([128, 4 * 64], f16, tag="rmat")
    nc.sync.dma_start(out=s_R[:], in_=d_R)
    s_hm = consts.tile([128, H * F], f32, tag="hm")
    nc.sync.dma_start(out=s_hm[:], in_=d_hm)
    s_id = consts.tile([128, 128], f16, tag="ident")
    nc.sync.dma_start(out=s_id[:], in_=d_id)
    # K=24 rhs for all groups: rows 0:16 maskinv, rows 16:24 alT02 replicated
    m24 = consts.tile([24, NGROUPS * N], f16, tag="m24")
    nc.sync.dma_start(out=m24[0:GSIZE, :], in_=d_minv)

    # ---- setup: projections ----
    # g_lT kept twice: f16 SBUF (DVE relu src, alT02 rhs) and fp32 PSUM
    # (ACT relu src: ScalarE's PSUM port is faster than its SBUF port)
    g_lT = consts.tile([H * F, N], f16, tag="glt")
    ps = ppe.tile([128, 2 * N], f32, tag="pe2")
    nc.tensor.matmul(ps[:, 0:N], s_WlT[:], s_hT[:], start=True, stop=True)
    nc.scalar.copy(g_lT[:], ps[:, 0:N])

    g_rT32 = consts.tile([H * F, N], f32, tag="grt32")
    ps = ppe.tile([128, 2 * N], f32, tag="pe2")
    nc.tensor.matmul(ps[:, 0:N], s_WrT[:], s_hT[:], start=True, stop=True)
    nc.vector.tensor_copy(g_rT32[:], ps[:, 0:N])

    # g_r natural layout + ones column per 128-row chunk (den for free)
    g_r_nat = consts.tile([128, 4 * 129], f16, tag="grnat")
    for c in range(4):
        cs = slice(128 * c, 128 * (c + 1))
        pq = pagg.tile([128, 258], f32, tag="agg")
        nc.tensor.matmul(pq[:, 0:128], s_hT[:, cs], s_WrT[:],
                         start=True, stop=True)
        nc.vector.tensor_copy(g_r_nat[:, 129 * c:129 * c + 128], pq[:, 0:128])
        nc.vector.memset(g_r_nat[:, 129 * c + 128:129 * c + 129], 1.0)

    # alT02[h,j] = 0.2*sum_f a_w[f]*g_l[j,(h,f)]; replicate x16 into m24 rows
    alT02 = consts.tile([H, N], f16, tag="alt02")
    ps = ppe.tile([128, 2 * N], f32, tag="pe2")
    nc.tensor.matmul(ps[0:H, 0:N], s_Aaw02[:], g_lT[:], start=True, stop=True)
    nc.scalar.copy(alT02[:], ps[0:H, 0:N])
    rep_ap = bass.AP(alT02.tensor, alT02[:].offset,
                     [alT02[:].ap[0], [0, NGROUPS], [1, N]])
    nc.sync.dma_start(out=m24[GSIZE:24, :], in_=rep_ap)

    # ---- main loop: software-pipelined pairs of 16-node groups ----
    # Stage p emits: exp for pair p-1 (ACT), transpose/copy/agg for pair
    # p-2 (PE+ACT), the score stage for pair p (DVE/ACT relus + PE MMs),
    # then recip/stt/R/out for pair p-2 (DVE+PE tails). Each engine's
    # strict-FIFO queue therefore never waits at its head on work another
    # engine issued in the same stage.
    order = [4 * s + r for r in range(4) for s in range(4)]
    NPAIRS = NGROUPS // 2
    e2s = {}
    um2s = {}
    umts = {}
    aggs = {}
    state = {"out_ps": None}

    def stage_exp(p):
        um2 = umpool.tile([128, 2 * N], f16, tag="um")
        nc.scalar.activation(um2[:], e2s.pop(p)[:], Exp)
        um2s[p] = um2

    def stage_transpose(p):
        um2 = um2s.pop(p)
        umt_ps = pumt.tile([128, 2 * N], f16, tag="umt")
        for k in range(8):
            ks = slice(128 * k, 128 * (k + 1))
            nc.tensor.transpose(umt_ps[:, ks], um2[:, ks], s_id[:])
        umt = umtp.tile([128, 2 * N], f16, tag="umtsb")
        nc.scalar.copy(umt[:], umt_ps[:])
        umts[p] = umt

    def stage_agg(p):
        umt = umts.pop(p)
        agg_ps = pagg.tile([128, 258], f32, tag="agg")
        for gg in range(2):
            for c in range(4):
                nc.tensor.matmul(agg_ps[:, 129 * gg:129 * gg + 129],
                                 umt[:, 512 * gg + 128 * c:
                                        512 * gg + 128 * (c + 1)],
                                 g_r_nat[:, 129 * c:129 * c + 129],
                                 start=(c == 0), stop=(c == 3))
        aggs[p] = agg_ps

    def stage_scores(p):
        e2 = ppe.tile([128, 2 * N], f32, tag="pe2")
        for gg in range(2):
            g = 2 * p + gg
            off = N * gg
            for idx, il in enumerate(order):
                i = GSIZE * g + il
                t_t = tpool.tile([H * F, N], f16, tag="t")
                if ENG_SCHED[idx] == "V":
                    nc.vector.tensor_scalar(t_t[:], g_lT[:],
                                            g_rT32[:, i:i + 1], 0.0, add, amax)
                else:
                    nc.scalar.activation(t_t[:], g_lT[:], Relu,
                                         bias=g_rT32[:, i:i + 1], scale=1.0)
                s = il // 4
                w0 = 24 - 8 * (il % 4)
                nc.tensor.matmul(e2[32 * s:32 * s + 32, off:off + N],
                                 s_Aaw[:, w0:w0 + 32], t_t[:],
                                 start=(idx < 4), stop=False,
                                 tile_position=(0, 32 * s),
                                 skip_group_check=True)
            # mask + alT linear term last: one K=24 matmul over all rows;
            # carries the stop flag so exp depends only on it
            nc.tensor.matmul(e2[:, off:off + N], s_m24w[:],
                             m24[:, N * g:N * (g + 1)],
                             start=False, stop=True, skip_group_check=True)
        e2s[p] = e2

    def stage_norm_out(p):
        agg_ps = aggs.pop(p)
        rd2 = denp.tile([128, 2], f32, tag="rden")
        den_ap = bass.AP(agg_ps.tensor, agg_ps[:, 128:129].offset,
                         [agg_ps[:, 128:129].ap[0], [129, 2], [1, 1]])
        nc.vector.reciprocal(rd2[:], den_ap)
        for gg in range(2):
            g = 2 * p + gg
            agg_sb = aggp.tile([128, D], f16, tag="aggsb")
            nc.vector.scalar_tensor_tensor(
                agg_sb[:], agg_ps[:, 129 * gg:129 * gg + 128],
                rd2[:, gg:gg + 1], s_hm[:], mult, mult)
            if g % 8 == 0:
                out_ps_new = pout.tile([128, D], f32, tag="out")
                state["out_ps"] = out_ps_new
            q = g % 4
            stripe = 64 * ((g % 8) // 4)
            out_ps = state["out_ps"]
            nc.tensor.matmul(out_ps[stripe:stripe + 64, :],
                             s_R[:, 64 * q:64 * q + 64], agg_sb[:],
                             start=(q == 0), stop=(q == 3))
            if q == 3:
                outb = outp.tile([64, D], f32, tag="outb")
                nc.vector.tensor_copy(outb[:], out_ps[stripe:stripe + 64, :])
                r0 = 64 * (g // 4)
                nc.sync.dma_start(out=d_out[r0:r0 + 64, :], in_=outb[:])

    for p in range(NPAIRS + 3):
        if p >= 1 and p - 1 < NPAIRS:
            stage_exp(p - 1)
        if p >= 2 and p - 2 < NPAIRS:
            stage_transpose(p - 2)
        if p >= 3:
            stage_agg(p - 3)
        if p < NPAIRS:
            stage_scores(p)
        if p >= 3:
            stage_norm_out(p - 3)


def _host_inputs(h, adj, W_l, W_r, a_w):
    """Build the per-core input maps (pure layout/constant prep)."""
    HF = H * F
    Aaw = np.zeros((HF, H), dtype=np.float32)
    for hh in range(H):
        Aaw[hh * F:(hh + 1) * F, hh] = a_w
    Amask = np.zeros((HF, 56), dtype=np.float32)
    Amask[:, 24:32] = (1.0 - NEG_SLOPE) * Aaw
    Amask = Amask.astype(np.float16)
    Aaw02 = (NEG_SLOPE * Aaw).astype(np.float16)
    m24w = np.zeros((24, 128), dtype=np.float32)
    for il in range(GSIZE):
        for hh in range(H):
            m24w[il, il * H + hh] = MASKPEN
            m24w[16 + hh, il * H + hh] = 1.0
    m24w = m24w.astype(np.float16)
    Rmask = np.zeros((128, 4 * 64), dtype=np.float16)
    for q in range(4):
        for il in range(GSIZE):
            Rmask[il * H:(il + 1) * H, 64 * q + 16 * q + il] = 1.0
    headmask = np.zeros((128, HF), dtype=np.float32)
    for il in range(GSIZE):
        for hh in range(H):
            headmask[il * H + hh, hh * F:(hh + 1) * F] = 1.0
    ident = np.eye(128, dtype=np.float16)
    WlT = np.ascontiguousarray(W_l.T).astype(np.float16)
    WrT = np.ascontiguousarray(W_r.T).astype(np.float16)

    in_maps = []
    for c in range(NCORES):
        b = c // 2
        i0 = IHALF * (c % 2)
        # Roll the node axis so this core's targets sit at 0..IHALF-1;
        # source order permuted consistently (softmax/agg j-invariant).
        maskseg = np.roll(adj[b, i0:i0 + IHALF, :, 0], -i0, axis=1)
        maskinv = np.ascontiguousarray(
            (1.0 - maskseg.reshape(NGROUPS, GSIZE, N).transpose(1, 0, 2)
             ).reshape(GSIZE, NGROUPS * N)).astype(np.float16)
        in_maps.append({
            "hT": np.ascontiguousarray(np.roll(h[b], -i0, axis=0).T).astype(
                np.float16),
            "WlT": WlT,
            "WrT": WrT,
            "Amask": Amask,
            "Aaw02": Aaw02,
            "m24w": m24w,
            "maskinv": maskinv,
            "Rmask": Rmask,
            "headmask": headmask,
            "ident": ident,
        })
    return in_maps


_NC_CACHE = {}
LAST_RESULT = None  # BassKernelResults of the most recent kernel() call


def _get_program():
    if "nc" not in _NC_CACHE:
        _NC_CACHE["nc"] = build_program()
    return _NC_CACHE["nc"]


def kernel(h, adj, W_l, W_r, a_w):
    h = np.asarray(h)
    adj = np.asarray(adj)
    W_l = np.asarray(W_l)
    W_r = np.asarray(W_r)
    a_w = np.asarray(a_w)

    nc = _get_program()
    in_maps = _host_inputs(h, adj, W_l, W_r, a_w)
    res = None
    for attempt in range(3):
        try:
            res = run_bass_kernel_spmd(nc, in_maps, list(range(NCORES)))
            break
        except Exception:
            # the axon-proxied device occasionally reports a transient
            # "unrecoverable" state at process start; it self-heals
            if attempt == 2:
                raise
            import time
            time.sleep(20)
    global LAST_RESULT
    LAST_RESULT = res

    out = np.zeros((B, N, D), dtype=np.float32)
    for c in range(NCORES):
        b = c // 2
        i0 = IHALF * (c % 2)
        out[b, i0:i0 + IHALF, :] = res.results[c]["out"]
    return out


# revision 20
# speedup vs baseline: 1.0296x; 1.0098x over previous
"""GATv2 layer kernel for Trainium2 (Bass/Tile), 8-core SPMD — v6.

Problem (hardcoded): B=4, N=512, D=128, H=8 heads, F=16 hidden, is_concat.
  g_l = h @ W_l.T ; g_r = h @ W_r.T               [B,N,H,F]
  e[b,i,j,h] = sum_f a_w[f]*lrelu(g_l[b,j,h,f] + g_r[b,i,h,f], 0.2)
  a = softmax_j(e masked by adj)                  [B,i,j,H]
  out[b,i,h,f] = sum_j a[b,i,j,h]*g_r[b,j,h,f]   -> [B,N,H*F]

Sharding: 8 cores = (batch b) x (i-half); 256 target nodes per core with
fully-replicated g_l/g_r.

Per-core math (rows (il*8+h), 16 groups of 16 i's):
  lrelu(x) = 0.8*relu(x) + 0.2*x; per-(i,h) j-constant terms cancel in
  softmax, so
    e[(il,h), j] = 0.8*sum_hf Aaw[hf,h]*relu(g_lT[hf,j] + g_rT[hf,i])
                   + 0.2*alT[h,j] - 44*(1-adj[i,j])    (+ const per row)
  The alT linear term AND the adjacency mask are applied by one K=24
  accumulating matmul (rhs = 16 maskinv rows + 8 alT rows); the -44
  penalty underflows exp() to exact 0 in fp16 (no post-exp mask multiply).

Software-pipelined by pairs: stage p emits exp for pair p-1, the
transpose/copy/agg for pair p-2, the score stage for pair p, then the
norm/out tail for pair p-2 — so no strict-FIFO engine queue waits at its
head on same-stage cross-engine work.

Score stage per group:
  t[hf,j] = relu(g_lT + g_rT[:,i]) - 12 on DVE (dual-op tensor_scalar,
            ~262ns) + 4 on ACT (activation relu+bias, ~620ns)
  e2      = 16 M=32 matmuls on 4 concurrent PE column strips
            (tile_position=(0,32s), per-strip start) + the K=24 mask/alT
            matmul last (carries the stop flag)
  um      = one exp over the pair's [128,1024] PSUM -> fp16       ACT
Per pair of groups:
  umT     = 8 PE transposes -> one fp16 PSUM bank [128,1024]
  umt_sb  = one ACT copy PSUM->SBUF
  aggT    = umt.T @ g_r_nat (4 K-chunks, ones col -> den)        PE
  agg_sb  = aggT * (1/den) * headmask                            DVE
  out     = R.T @ agg_sb (4 groups per 64-row stripe)            PE
"""

import ml_dtypes
import numpy as np
from contextlib import ExitStack

import concourse.bass as bass
import concourse.bacc as bacc
import concourse.tile as tile
import concourse.mybir as mybir
from concourse.bass_utils import run_bass_kernel_spmd

B, N, D = 4, 512, 128
H, F = 8, 16
NEG_SLOPE = 0.2
NCORES = 8
IHALF = N // 2          # 256 target nodes per core
GSIZE = 16              # i's per group
NGROUPS = IHALF // GSIZE  # 16
MASKPEN = -44.0
f32 = mybir.dt.float32
f16 = mybir.dt.float16

# Engine per relu slot (program order = strip-round-robin il order):
# V = VectorE dual-op tensor_scalar, A = ScalarE activation relu.
ENG_SCHED = "AVVVAVVVAVVVAVVV"


def build_program():
    nc = bacc.Bacc(
        "TRN2", target_bir_lowering=False, debug=False, num_devices=NCORES
    )

    d_hT = nc.dram_tensor("hT", [D, N], f16, kind="ExternalInput").ap()
    d_WlT = nc.dram_tensor("WlT", [D, H * F], f16, kind="ExternalInput").ap()
    d_WrT = nc.dram_tensor("WrT", [D, H * F], f16, kind="ExternalInput").ap()
    # Amask[:, 24:32] = 0.8*Aaw; LDW windows [24-8m, 56-8m) place node
    # (il%4)'s head-block at strip rows 8m..8m+8.
    d_Aaw = nc.dram_tensor("Amask", [H * F, 56], f16, kind="ExternalInput").ap()
    d_Aaw02 = nc.dram_tensor("Aaw02", [H * F, H], f16, kind="ExternalInput").ap()
    d_m24w = nc.dram_tensor("m24w", [24, 128], f16, kind="ExternalInput").ap()
    d_minv = nc.dram_tensor("maskinv", [GSIZE, NGROUPS * N], f16,
                            kind="ExternalInput").ap()
    d_R = nc.dram_tensor("Rmask", [128, 4 * 64], f16, kind="ExternalInput").ap()
    d_hm = nc.dram_tensor("headmask", [128, H * F], f32, kind="ExternalInput").ap()
    d_id = nc.dram_tensor("ident", [128, 128], f16, kind="ExternalInput").ap()
    d_out = nc.dram_tensor("out", [IHALF, D], f32, kind="ExternalOutput").ap()

    with tile.TileContext(nc) as tc:
        with ExitStack() as ctx:
            _gat_body(ctx, tc, d_out, d_hT, d_WlT, d_WrT, d_Aaw, d_Aaw02,
                      d_m24w, d_minv, d_R, d_hm, d_id)
    nc.compile()
    return nc


def _gat_body(ctx, tc, d_out, d_hT, d_WlT, d_WrT, d_Aaw, d_Aaw02, d_m24w,
              d_minv, d_R, d_hm, d_id):
    nc = tc.nc
    add = mybir.AluOpType.add
    amax = mybir.AluOpType.max
    mult = mybir.AluOpType.mult
    Relu = mybir.ActivationFunctionType.Relu
    Exp = mybir.ActivationFunctionType.Exp

    consts = ctx.enter_context(tc.tile_pool(name="consts", bufs=1))
    tpool = ctx.enter_context(tc.tile_pool(name="tpool", bufs=28))
    umpool = ctx.enter_context(tc.tile_pool(name="umpool", bufs=5))
    umtp = ctx.enter_context(tc.tile_pool(name="umtp", bufs=4))
    aggp = ctx.enter_context(tc.tile_pool(name="aggp", bufs=4))
    denp = ctx.enter_context(tc.tile_pool(name="denp", bufs=4))
    outp = ctx.enter_context(tc.tile_pool(name="outp", bufs=2))

    # PSUM budget (8 banks): ppe 2x[128,1024]f32 (4) + pumt 1x[128,1024]f16
    # (1) + pagg 2x[128,258]f32 (2) + pout 1x[128,128]f32 (1)
    ppe = ctx.enter_context(tc.tile_pool(name="ppe", bufs=2, space="PSUM"))
    pumt = ctx.enter_context(tc.tile_pool(name="pumt", bufs=1, space="PSUM"))
    pagg = ctx.enter_context(tc.tile_pool(name="pagg", bufs=1, space="PSUM"))
    pout = ctx.enter_context(tc.tile_pool(name="pout", bufs=1, space="PSUM"))

    # ---- ACT table warm-up (overlaps the input DMAs) ----
    warm = consts.tile([128, 8], f32, tag="warm")
    nc.vector.memset(warm[:], 0.0)
    nc.scalar.activation(warm[:], warm[:], Exp)

    # ---- load constants ----
    s_hT = consts.tile([D, N], f16, tag="ht")
    nc.sync.dma_start(out=s_hT[:], in_=d_hT)
    s_WlT = consts.tile([D, H * F], f16, tag="wlt")
    nc.sync.dma_start(out=s_WlT[:], in_=d_WlT)
    s_WrT = consts.tile([D, H * F], f16, tag="wrt")
    nc.sync.dma_start(out=s_WrT[:], in_=d_WrT)
    s_Aaw = consts.tile([H * F, 56], f16, tag="aaw")
    nc.scalar.dma_start(out=s_Aaw[:], in_=d_Aaw)
    s_Aaw02 = consts.tile([H * F, H], f16, tag="aaw02")
    nc.scalar.dma_start(out=s_Aaw02[:], in_=d_Aaw02)
    s_m24w = consts.tile([24, 128], f16, tag="m24w")
    nc.scalar.dma_start(out=s_m24w[:], in_=d_m24w)
    s_R = consts.tile([128, 4 * 64], f16, tag="rmat")
    nc.sync.dma_start(out=s_R[:], in_=d_R)
    s_hm = consts.tile([128, H * F], f32, tag="hm")
    nc.sync.dma_start(out=s_hm[:], in_=d_hm)
    s_id = consts.tile([128, 128], f16, tag="ident")
    nc.sync.dma_start(out=s_id[:], in_=d_id)
    # K=24 rhs for all groups: rows 0:16 maskinv, rows 16:24 alT02 replicated
    m24 = consts.tile([24, NGROUPS * N], f16, tag="m24")
    nc.sync.dma_start(out=m24[0:GSIZE, :], in_=d_minv)

    # ---- setup: projections ----
    # g_lT kept twice: f16 SBUF (DVE relu src, alT02 rhs) and fp32 PSUM
    # (ACT relu src: ScalarE's PSUM port is faster than its SBUF port)
    g_lT = consts.tile([H * F, N], f16, tag="glt")
    ps = ppe.tile([128, 2 * N], f32, tag="pe2")
    nc.tensor.matmul(ps[:, 0:N], s_WlT[:], s_hT[:], start=True, stop=True)
    nc.scalar.copy(g_lT[:], ps[:, 0:N])

    g_rT32 = consts.tile([H * F, N], f32, tag="grt32")
    ps = ppe.tile([128, 2 * N], f32, tag="pe2")
    nc.tensor.matmul(ps[:, 0:N], s_WrT[:], s_hT[:], start=True, stop=True)
    nc.vector.tensor_copy(g_rT32[:], ps[:, 0:N])

    # g_r natural layout + ones column per 128-row chunk (den for free)
    g_r_nat = consts.tile([128, 4 * 129], f16, tag="grnat")
    for c in range(4):
        cs = slice(128 * c, 128 * (c + 1))
        pq = pagg.tile([128, 258], f32, tag="agg")
        nc.tensor.matmul(pq[:, 0:128], s_hT[:, cs], s_WrT[:],
                         start=True, stop=True)
        nc.vector.tensor_copy(g_r_nat[:, 129 * c:129 * c + 128], pq[:, 0:128])
        nc.vector.memset(g_r_nat[:, 129 * c + 128:129 * c + 129], 1.0)

    # alT02[h,j] = 0.2*sum_f a_w[f]*g_l[j,(h,f)]; replicate x16 into m24 rows
    alT02 = consts.tile([H, N], f16, tag="alt02")
    ps = ppe.tile([128, 2 * N], f32, tag="pe2")
    nc.tensor.matmul(ps[0:H, 0:N], s_Aaw02[:], g_lT[:], start=True, stop=True)
    nc.scalar.copy(alT02[:], ps[0:H, 0:N])
    rep_ap = bass.AP(alT02.tensor, alT02[:].offset,
                     [alT02[:].ap[0], [0, NGROUPS], [1, N]])
    nc.sync.dma_start(out=m24[GSIZE:24, :], in_=rep_ap)

    # ---- main loop: software-pipelined pairs of 16-node groups ----
    # Stage p emits: exp for pair p-1 (ACT), transpose/copy/agg for pair
    # p-2 (PE+ACT), the score stage for pair p (DVE/ACT relus + PE MMs),
    # then recip/stt/R/out for pair p-2 (DVE+PE tails). Each engine's
    # strict-FIFO queue therefore never waits at its head on work another
    # engine issued in the same stage.
    order = [4 * s + r for r in range(4) for s in range(4)]
    NPAIRS = NGROUPS // 2
    e2s = {}
    um2s = {}
    umts = {}
    aggs = {}
    state = {"out_ps": None}

    def stage_exp(p):
        um2 = umpool.tile([128, 2 * N], f16, tag="um")
        nc.scalar.activation(um2[:], e2s.pop(p)[:], Exp)
        um2s[p] = um2

    def stage_transpose(p):
        um2 = um2s.pop(p)
        umt_ps = pumt.tile([128, 2 * N], f16, tag="umt")
        for k in range(8):
            ks = slice(128 * k, 128 * (k + 1))
            nc.tensor.transpose(umt_ps[:, ks], um2[:, ks], s_id[:])
        umt = umtp.tile([128, 2 * N], f16, tag="umtsb")
        nc.scalar.copy(umt[:], umt_ps[:])
        umts[p] = umt

    def stage_agg(p):
        umt = umts.pop(p)
        agg_ps = pagg.tile([128, 258], f32, tag="agg")
        for gg in range(2):
            for c in range(4):
                nc.tensor.matmul(agg_ps[:, 129 * gg:129 * gg + 129],
                                 umt[:, 512 * gg + 128 * c:
                                        512 * gg + 128 * (c + 1)],
                                 g_r_nat[:, 129 * c:129 * c + 129],
                                 start=(c == 0), stop=(c == 3))
        aggs[p] = agg_ps

    def stage_scores(p):
        e2 = ppe.tile([128, 2 * N], f32, tag="pe2")
        for gg in range(2):
            g = 2 * p + gg
            off = N * gg
            for idx, il in enumerate(order):
                i = GSIZE * g + il
                t_t = tpool.tile([H * F, N], f16, tag="t")
                if ENG_SCHED[idx] == "V":
                    nc.vector.tensor_scalar(t_t[:], g_lT[:],
                                            g_rT32[:, i:i + 1], 0.0, add, amax)
                else:
                    nc.scalar.activation(t_t[:], g_lT[:], Relu,
                                         bias=g_rT32[:, i:i + 1], scale=1.0)
                s = il // 4
                w0 = 24 - 8 * (il % 4)
                nc.tensor.matmul(e2[32 * s:32 * s + 32, off:off + N],
                                 s_Aaw[:, w0:w0 + 32], t_t[:],
                                 start=(idx < 4), stop=False,
                                 tile_position=(0, 32 * s),
                                 skip_group_check=True)
            # mask + alT linear term last: one K=24 matmul over all rows;
            # carries the stop flag so exp depends only on it
            nc.tensor.matmul(e2[:, off:off + N], s_m24w[:],
                             m24[:, N * g:N * (g + 1)],
                             start=False, stop=True, skip_group_check=True)
        e2s[p] = e2

    def stage_norm_out(p):
        agg_ps = aggs.pop(p)
        rd2 = denp.tile([128, 2], f32, tag="rden")
        den_ap = bass.AP(agg_ps.tensor, agg_ps[:, 128:129].offset,
                         [agg_ps[:, 128:129].ap[0], [129, 2], [1, 1]])
        nc.vector.reciprocal(rd2[:], den_ap)
        for gg in range(2):
            g = 2 * p + gg
            agg_sb = aggp.tile([128, D], f16, tag="aggsb")
            nc.vector.scalar_tensor_tensor(
                agg_sb[:], agg_ps[:, 129 * gg:129 * gg + 128],
                rd2[:, gg:gg + 1], s_hm[:], mult, mult)
            if g % 8 == 0:
                out_ps_new = pout.tile([128, D], f32, tag="out")
                state["out_ps"] = out_ps_new
            q = g % 4
            stripe = 64 * ((g % 8) // 4)
            out_ps = state["out_ps"]
            nc.tensor.matmul(out_ps[stripe:stripe + 64, :],
                             s_R[:, 64 * q:64 * q + 64], agg_sb[:],
                             start=(q == 0), stop=(q == 3))
            if q == 3:
                outb = outp.tile([64, D], f32, tag="outb")
                nc.vector.tensor_copy(outb[:], out_ps[stripe:stripe + 64, :])
                r0 = 64 * (g // 4)
                nc.sync.dma_start(out=d_out[r0:r0 + 64, :], in_=outb[:])

    for p in range(NPAIRS + 3):
        if p >= 1 and p - 1 < NPAIRS:
            stage_exp(p - 1)
        if p >= 2 and p - 2 < NPAIRS:
            stage_transpose(p - 2)
        if p >= 3:
            stage_agg(p - 3)
        if p < NPAIRS:
            stage_scores(p)
        if p >= 3:
            stage_norm_out(p - 3)


def _host_inputs(h, adj, W_l, W_r, a_w):
    """Build the per-core input maps (pure layout/constant prep)."""
    HF = H * F
    Aaw = np.zeros((HF, H), dtype=np.float32)
    for hh in range(H):
        Aaw[hh * F:(hh + 1) * F, hh] = a_w
    Amask = np.zeros((HF, 56), dtype=np.float32)
    Amask[:, 24:32] = (1.0 - NEG_SLOPE) * Aaw
    Amask = Amask.astype(np.float16)
    Aaw02 = (NEG_SLOPE * Aaw).astype(np.float16)
    m24w = np.zeros((24, 128), dtype=np.float32)
    for il in range(GSIZE):
        for hh in range(H):
            m24w[il, il * H + hh] = MASKPEN
            m24w[16 + hh, il * H + hh] = 1.0
    m24w = m24w.astype(np.float16)
    Rmask = np.zeros((128, 4 * 64), dtype=np.float16)
    for q in range(4):
        for il in range(GSIZE):
            Rmask[il * H:(il + 1) * H, 64 * q + 16 * q + il] = 1.0
    headmask = np.zeros((128, HF), dtype=np.float32)
    for il in range(GSIZE):
        for hh in range(H):
            headmask[il * H + hh, hh * F:(hh + 1) * F] = 1.0
    ident = np.eye(128, dtype=np.float16)
    WlT = np.ascontiguousarray(W_l.T).astype(np.float16)
    WrT = np.ascontiguousarray(W_r.T).astype(np.float16)

    in_maps = []
    for c in range(NCORES):
        b = c // 2
        i0 = IHALF * (c % 2)
        # Roll the node axis so this core's targets sit at 0..IHALF-1;
        # source order permuted consistently (softmax/agg j-invariant).
        maskseg = np.roll(adj[b, i0:i0 + IHALF, :, 0], -i0, axis=1)
        maskinv = np.ascontiguousarray(
            (1.0 - maskseg.reshape(NGROUPS, GSIZE, N).transpose(1, 0, 2)
             ).reshape(GSIZE, NGROUPS * N)).astype(np.float16)
        in_maps.append({
            "hT": np.ascontiguousarray(np.roll(h[b], -i0, axis=0).T).astype(
                np.float16),
            "WlT": WlT,
            "WrT": WrT,
            "Amask": Amask,
            "Aaw02": Aaw02,
            "m24w": m24w,
            "maskinv": maskinv,
            "Rmask": Rmask,
            "headmask": headmask,
            "ident": ident,
        })
    return in_maps


_NC_CACHE = {}
LAST_RESULT = None  # BassKernelResults of the most recent kernel() call


def _get_program():
    if "nc" not in _NC_CACHE:
        _NC_CACHE["nc"] = build_program()
    return _NC_CACHE["nc"]


def kernel(h, adj, W_l, W_r, a_w):
    h = np.asarray(h)
    adj = np.asarray(adj)
    W_l = np.asarray(W_l)
    W_r = np.asarray(W_r)
    a_w = np.asarray(a_w)

    nc = _get_program()
    in_maps = _host_inputs(h, adj, W_l, W_r, a_w)
    res = None
    for attempt in range(3):
        try:
            res = run_bass_kernel_spmd(nc, in_maps, list(range(NCORES)))
            break
        except Exception:
            # the axon-proxied device occasionally reports a transient
            # "unrecoverable" state at process start; it self-heals
            if attempt == 2:
                raise
            import time
            time.sleep(20)
    global LAST_RESULT
    LAST_RESULT = res

    out = np.zeros((B, N, D), dtype=np.float32)
    for c in range(NCORES):
        b = c // 2
        i0 = IHALF * (c % 2)
        out[b, i0:i0 + IHALF, :] = res.results[c]["out"]
    return out
